# revision 7
# baseline (speedup 1.0000x reference)
"""Trainium2 Bass kernel for nn_CausalFT (causal Fourier transform + residual + LayerNorm).

reference semantics (QLEN=1024, MLEN=1024, BATCH=8, D_MODEL=1024, klen=2048):
    cat  = concat([mems, dec_inp], axis=0) (+ pos_emb broadcast over batch)
    ft   = einsum('ml,lbd->mbd', ft_matrix(1024, 2048), cat)
    x    = dec_inp + ft / sqrt(2048)
    out  = LayerNorm_d(x) * gamma + beta

Sharding: data-parallel over batch — core b computes out[:, b, :] entirely
(no collectives).

Core trick: LayerNorm is scale-invariant, so the kernel computes
    psum = c*W @ cat + (c*sqrt(klen)) * dec        (c = 360/sqrt(klen))
and normalizes psum directly — the 1/sqrt(klen) is never applied and the
residual add runs on the PE as a diagonal bf16 matmul (360 is exact in
bf16; LN_EPS is pre-scaled by 360^2 to compensate).

The banded FT matrix (ft[m,j] != 0 only for m <= j <= m+1024) needs 9
128-wide contraction tiles per 128-row output tile.  8 of them run as 4
fp8 DoubleRow matmuls (2 contraction rows/PE cell/cycle), the 9th as a
plain fp8 matmul, plus the bf16 diagonal for the residual.

All inputs ship partition-major ([p, k, d] with p the SBUF partition) so
every DMA is a maximal contiguous copy.  fp8 quantization of the FT
channel is harmless: it enters the output scaled by 1/sqrt(klen) (~2% of
the residual's scale), measured output rel err ~2e-4 vs the 2e-2 gate.
"""

import math

import numpy as np

QLEN, MLEN, BATCH, D = 1024, 1024, 8, 1024
KLEN = QLEN + MLEN
NT = QLEN // 128          # 8 output row tiles
NK = KLEN // 128          # 16 contraction tiles
LN_EPS = 1e-5

IDENT_VAL = 360.0                       # exact in bf16
WSCALE = IDENT_VAL / math.sqrt(KLEN)    # weight prescale; psum = 360 * x
EPS_SCALED = LN_EPS * IDENT_VAL * IDENT_VAL
N_WARM = 40                             # PE warmup matmuls (HAM clock-gate)

_CONST = None
_PROGS = {}


def _ft_matrix_np():
    """Replicate reference._ft_matrix bit-for-bit using jax on CPU."""
    import jax
    import jax.numpy as jnp

    cpu = jax.local_devices(backend="cpu")[0]
    with jax.default_device(cpu):
        qlen, klen = QLEN, KLEN
        ft_len = klen - qlen + 1
        m = jnp.arange(qlen, dtype=jnp.float32)
        k = jnp.arange(ft_len, dtype=jnp.float32)
        base = jnp.cos((2.0 * float(np.pi)) * jnp.outer(m, k) / float(ft_len))
        base = base / float(np.sqrt(ft_len))
        mat = jnp.pad(base, ((0, 0), (klen - ft_len, 0)))
        shift = (qlen - 1) - jnp.arange(qlen)
        cols = (jnp.arange(klen)[None, :] + shift[:, None]) % klen
        mat = jnp.take_along_axis(mat, cols, axis=1)
        rows = jnp.arange(qlen)[:, None]
        js = jnp.arange(klen)[None, :]
        mask = (js <= rows + (klen - qlen)) & (js >= rows)
        mat = jnp.where(mask, mat, jnp.float32(0.0))
        return np.asarray(jax.device_get(mat), dtype=np.float32)


def _f8(x):
    import ml_dtypes

    return np.clip(x, -240.0, 240.0).astype(ml_dtypes.float8_e4m3)


def _bf16(x):
    import ml_dtypes

    return np.asarray(x).astype(ml_dtypes.bfloat16)


def _consts():
    """Pre-packed weight constants: (wall, ident).

    wall[p, t, i, m] = Wc[128t+m, 128(t+i)+p]  (fp8; i=0..7 DoubleRow pairs
    as (2i, 2i+1), i=8 the single trailing band block)
    ident            = 360 * I[128]            (bf16)
    where Wc = ft_matrix * (360/sqrt(klen)).
    """
    global _CONST
    if _CONST is None:
        w = _ft_matrix_np() * np.float32(WSCALE)
        wall = np.empty((128, NT, 9, 128), dtype=np.float32)
        for t in range(NT):
            for i in range(9):
                k = t + i
                blk = w[128 * t : 128 * (t + 1), 128 * k : 128 * k + 128]
                wall[:, t, i, :] = blk.T
        ident = np.eye(128, dtype=np.float32) * np.float32(IDENT_VAL)
        _CONST = (
            np.ascontiguousarray(_f8(wall)),
            np.ascontiguousarray(_bf16(ident)),
        )
    return _CONST


def _install_drain_patch():
    """Work around walrus 'Too many sync wait commands' on the Tile tail drain.

    The stock TileContext._drain_and_barrier emits ONE sync-engine Drain
    carrying a sem wait for every proc lane that ticked (up to 27).  The
    walrus build in this environment accepts only a single sync-wait per
    instruction, so split the global-clock wait set across consecutive
    Drains (one wait each) — sequential execution on the same engine gives
    the same quiescence guarantee.  Also skip the tail per-sem zeroing:
    the bass preamble range-clears every kernel semaphore at program start
    on each execution, so the ~250 walrus-expanded tail EVSEMs (~8us) are
    redundant for re-execution correctness.
    """
    import re

    import bass_rust
    import concourse.tile as _tile
    from concourse.vector_clock import ScopedClock

    if getattr(_tile.TileContext, "_drain_patch_installed", False):
        return

    def _clock_ticks(vc):
        m = re.search(r"\[([0-9, ]*)\]", repr(vc))
        if not m or not m.group(1).strip():
            return []
        return [int(x) for x in m.group(1).split(",")]

    def _patched_drain_and_barrier(self, tick_clock, wait_clock):
        nc = self.nc
        ticks = _clock_ticks(tick_clock.global_clock)
        for i, t in enumerate(ticks):
            if t > 0:
                part = bass_rust.VectorClock()
                part.require_at_least(i, t)
                d = nc.sync.drain()
                wait_clock.add_sem_waits(d.ins, ScopedClock({None: part}))
        assert self.sems is not None
        popped = nc._tile_sem_poison_stack.pop()
        assert popped is self._sem_poison
        nc._state.prepend_free_semaphores(
            [s.num for s in self.sems.allocated().values()]
        )

    _tile.TileContext._drain_and_barrier = _patched_drain_and_barrier
    _tile.TileContext._drain_patch_installed = True


def _split_excess_waits(nc, cap=1):
    """Hoist excess per-instruction sem waits onto preceding same-engine nops.

    The walrus build here accepts only `cap` sync-wait commands per
    instruction.  Engines execute their instruction stream in order, so
    moving waits to immediately-preceding same-engine nops preserves the
    ordering semantics (the instruction still starts only after every wait
    is satisfied).
    """
    import concourse.mybir as mybir

    for bb in nc.main_func.blocks:
        insts = list(bb.instructions)
        if not any(
            i.sync_info and i.sync_info.on_wait and len(i.sync_info.on_wait) > cap
            for i in insts
        ):
            continue
        new = []
        for inst in insts:
            si = inst.sync_info
            waits = list(si.on_wait) if si and si.on_wait else []
            if len(waits) > cap:
                for sw in waits[:-cap]:
                    nop = nc.engines[inst.engine].nop(nofuse=True).ins
                    cur = nc.cur_bb.bb
                    assert cur.instructions and cur.instructions[-1] is nop
                    cur.instructions.pop()
                    nop.sync_info = mybir.SyncInfo(on_wait=[sw], on_update=[])
                    new.append(nop)
                inst.sync_info = mybir.SyncInfo(
                    on_wait=waits[-cap:], on_update=list(si.on_update or [])
                )
            new.append(inst)
        bb.instructions.clear()
        for i in new:
            bb.instructions.append(i)


def _build_program(add_pos: bool, trivial_affine: bool):
    _install_drain_patch()
    import concourse.bass as bass
    import concourse.mybir as mybir
    import concourse.tile as tile

    f32 = mybir.dt.float32
    bf16 = mybir.dt.bfloat16
    f8 = mybir.dt.float8e4
    DR = mybir.MatmulPerfMode.DoubleRow
    nc = bass.Bass()

    # All I/O partition-major: [p, tile, d] so DMAs are contiguous copies.
    memsd = nc.dram_tensor("memsd", [128, NT, D], f8, kind="ExternalInput")
    decd = nc.dram_tensor("decd", [128, NT, D], bf16, kind="ExternalInput")
    posd = None
    if add_pos:
        posd = nc.dram_tensor("posd", [128, NK, D], f8, kind="ExternalInput")
    walld = nc.dram_tensor("walld", [128, NT, 9, 128], f8, kind="ExternalInput")
    identd = nc.dram_tensor("identd", [128, 128], bf16, kind="ExternalInput")
    gamd = betd = None
    if not trivial_affine:
        gamd = nc.dram_tensor("gamd", [D], f32, kind="ExternalInput")
        betd = nc.dram_tensor("betd", [D], f32, kind="ExternalInput")
    outd = nc.dram_tensor("outd", [128, NT, D], bf16, kind="ExternalOutput")

    with tile.TileContext(nc) as tc:
        with (
            tc.tile_pool(name="big", bufs=1) as big,
            tc.tile_pool(name="stat", bufs=4) as stat,
            tc.tile_pool(name="ps", bufs=3, space="PSUM") as ps,
            tc.tile_pool(name="psw", bufs=1, space="PSUM") as psw,
        ):
            eps = big.tile([128, 1], f32)
            nc.vector.memset(eps, EPS_SCALED)

            id_sb = big.tile([128, 128], bf16)
            wa_sb = big.tile([128, NT, 9, 128], f8)
            mems_sb = big.tile([128, NT, D], f8)
            dec_sb = big.tile([128, NT, D], bf16)
            cat8 = big.tile([128, NK, D], f8)
            pos_sb = None
            if add_pos:
                pos_sb = big.tile([128, NK, D], f8, name="pos_sb")
            o_sb = big.tile([128, NT, D], bf16)
            gam_sb = bet_sb = None
            if not trivial_affine:
                gam_sb = big.tile([128, D], f32)
                bet_sb = big.tile([128, D], f32)
                gam_ap, bet_ap = gamd[:], betd[:]
                nc.sync.dma_start(
                    out=gam_sb,
                    in_=bass.AP(tensor=gam_ap.tensor, offset=0, ap=[[0, 128]] + list(gam_ap.ap)),
                )
                nc.sync.dma_start(
                    out=bet_sb,
                    in_=bass.AP(tensor=bet_ap.tensor, offset=0, ap=[[0, 128]] + list(bet_ap.ap)),
                )

            # --- DMA pushes on three rings (sync/scalar/gpsimd), ordered so
            # the bytes that gate group 0 (ident, wall[0], dec[0], all mems,
            # pos[0:9]) stream first and roughly evenly across rings. ---
            S, C, G = nc.sync.dma_start, nc.scalar.dma_start, nc.gpsimd.dma_start
            S(out=id_sb, in_=identd[:, :])
            S(out=wa_sb[:, 0, :, :], in_=walld[:, 0, :, :])
            S(out=dec_sb[:, 0:1, :], in_=decd[:, 0:1, :])
            S(out=mems_sb[:, 0:2, :], in_=memsd[:, 0:2, :])
            if add_pos:
                C(out=pos_sb[:, 0:4, :], in_=posd[:, 0:4, :])
                G(out=pos_sb[:, 4:7, :], in_=posd[:, 4:7, :])
                S(out=pos_sb[:, 7:9, :], in_=posd[:, 7:9, :])
            C(out=mems_sb[:, 6:8, :], in_=memsd[:, 6:8, :])
            G(out=mems_sb[:, 2:6, :], in_=memsd[:, 2:6, :])
            S(out=dec_sb[:, 1:4, :], in_=decd[:, 1:4, :])
            C(out=wa_sb[:, 1:4, :, :], in_=walld[:, 1:4, :, :])
            if add_pos:
                C(out=pos_sb[:, 9:12, :], in_=posd[:, 9:12, :])
            S(out=wa_sb[:, 4:NT, :, :], in_=walld[:, 4:NT, :, :])
            G(out=dec_sb[:, 4:NT, :], in_=decd[:, 4:NT, :])
            if add_pos:
                C(out=pos_sb[:, 12:NK, :], in_=posd[:, 12:NK, :])

            # --- PE warmup: the HAM clock gate needs ~3.4us of sustained
            # busy before the PE runs at 2.4 GHz instead of 1.2.  Burn the
            # DMA ramp on dummy matmuls so the real ones start warm. ---
            warm = psw.tile([128, 128], f32)
            for _ in range(N_WARM):
                nc.tensor.matmul(warm, id_sb, id_sb, start=True, stop=True)

            # --- cat tiles: cat8[k] = (mems|dec)[k] + pos[k], fp8 out.
            # fp8 tensor_tensor runs 1x on DVE (~1.2us/tile): mems-side adds
            # go on DVE, dec-side adds on the otherwise-idle GpSimd
            # (~2.2us/tile), one per group, so neither engine serializes the
            # per-group epilogues. ---
            def emit_add(k):
                eng = nc.vector if k < NT else nc.gpsimd
                if add_pos:
                    if k < NT:
                        eng.tensor_add(
                            out=cat8[:, k, :], in0=mems_sb[:, k, :], in1=pos_sb[:, k, :]
                        )
                    else:
                        eng.tensor_add(
                            out=cat8[:, k, :],
                            in0=dec_sb[:, k - NT, :],
                            in1=pos_sb[:, k, :],
                        )
                else:
                    if k < NT:
                        eng.tensor_copy(out=cat8[:, k, :], in_=mems_sb[:, k, :])
                    else:
                        eng.tensor_copy(out=cat8[:, k, :], in_=dec_sb[:, k - NT, :])

            for k in range(9):
                emit_add(k)

            # --- per output row tile: 12 matmuls into two psum banks, then
            # LN epilogue straight off psum ---
            for t in range(NT):
                if t + 9 < NK:
                    emit_add(t + 9)
                psA = ps.tile([128, 512], f32, tag="A")
                psB = ps.tile([128, 512], f32, tag="B")
                for h, pst in ((0, psA), (1, psB)):
                    sl = slice(512 * h, 512 * (h + 1))
                    nc.tensor.matmul(
                        pst, id_sb, dec_sb[:, t, sl], start=True, stop=False
                    )
                    for i in range(4):
                        nc.tensor.matmul(
                            pst,
                            wa_sb[:, t, 2 * i : 2 * i + 2, :],
                            cat8[:, t + 2 * i : t + 2 * i + 2, sl],
                            start=False,
                            stop=False,
                            perf_mode=DR,
                        )
                    nc.tensor.matmul(
                        pst, wa_sb[:, t, 8, :], cat8[:, t + 8, sl], start=False, stop=True
                    )

                st = stat.tile([128, 2, 6], f32, tag="st")
                nc.vector.bn_stats(out=st[:, 0, :], in_=psA)
                nc.vector.bn_stats(out=st[:, 1, :], in_=psB)
                mv = stat.tile([128, 2], f32, tag="mv")
                nc.vector.bn_aggr(out=mv, in_=st)
                rs = stat.tile([128, 1], f32, tag="rs")
                nc.scalar.activation(
                    out=rs, in_=mv[:, 1:2],
                    func=mybir.ActivationFunctionType.Sqrt,
                    bias=eps, scale=1.0,
                )
                nc.vector.reciprocal(out=rs, in_=rs)
                negms = stat.tile([128, 1], f32, tag="negms")
                nc.vector.tensor_scalar(
                    out=negms, in0=mv[:, 0:1], scalar1=rs, scalar2=-1.0,
                    op0=mybir.AluOpType.mult, op1=mybir.AluOpType.mult,
                )
                # both apply halves on ACT (DVE is the busier engine); for
                # the last groups split A/B across ACT+DVE to shorten the tail
                nc.scalar.activation(
                    out=o_sb[:, t, 0:512], in_=psA,
                    func=mybir.ActivationFunctionType.Identity,
                    bias=negms, scale=rs,
                )
                if t >= NT - 2:
                    nc.vector.tensor_scalar(
                        out=o_sb[:, t, 512:1024], in0=psB,
                        scalar1=mv[:, 0:1], scalar2=rs,
                        op0=mybir.AluOpType.subtract, op1=mybir.AluOpType.mult,
                    )
                else:
                    nc.scalar.activation(
                        out=o_sb[:, t, 512:1024], in_=psB,
                        func=mybir.ActivationFunctionType.Identity,
                        bias=negms, scale=rs,
                    )
                if not trivial_affine:
                    nc.vector.tensor_mul(out=o_sb[:, t, :], in0=o_sb[:, t, :], in1=gam_sb)
                    nc.vector.tensor_add(out=o_sb[:, t, :], in0=o_sb[:, t, :], in1=bet_sb)
                # stores alternate the sync/scalar rings (gpsimd is doing adds)
                (S if t % 2 == 0 else C)(out=outd[:, t, :], in_=o_sb[:, t, :])

    _split_excess_waits(nc)
    return nc


def _get_program(add_pos: bool, trivial_affine: bool):
    key = (add_pos, trivial_affine)
    if key not in _PROGS:
        _PROGS[key] = _build_program(add_pos, trivial_affine)
    return _PROGS[key]


def _part_major(x2d, ntile):
    """[ntile*128, D] -> [128, ntile, D] (partition-major)."""
    return np.ascontiguousarray(
        x2d.reshape(ntile, 128, x2d.shape[1]).transpose(1, 0, 2)
    )


def _make_in_maps(dec_inp, pos_emb, mems, gamma, beta, add_pos, trivial):
    wall, ident = _consts()
    pos_r = None
    if add_pos:
        pos_r = _f8(_part_major(np.ascontiguousarray(pos_emb[:, 0, :]), NK))
    in_maps = []
    for b in range(BATCH):
        m = {
            "memsd": _f8(_part_major(np.ascontiguousarray(mems[:, b, :]), NT)),
            "decd": _bf16(_part_major(np.ascontiguousarray(dec_inp[:, b, :]), NT)),
            "walld": wall,
            "identd": ident,
        }
        if add_pos:
            m["posd"] = pos_r
        if not trivial:
            m["gamd"] = np.asarray(gamma, dtype=np.float32)
            m["betd"] = np.asarray(beta, dtype=np.float32)
        in_maps.append(m)
    return in_maps


def _unpack_out(res):
    """results[b]['outd'] [128, NT, D] bf16 -> [QLEN, BATCH, D] fp32."""
    outs = []
    for b in range(BATCH):
        o = np.asarray(res.results[b]["outd"]).astype(np.float32)
        outs.append(o.transpose(1, 0, 2).reshape(QLEN, D))
    return np.stack(outs, axis=1)


def kernel(dec_inp, pos_emb, mems, gamma, beta, add_position):
    from concourse.bass_utils import run_bass_kernel_spmd

    dec_inp = np.asarray(dec_inp, dtype=np.float32)
    pos_emb = np.asarray(pos_emb, dtype=np.float32)
    mems = np.asarray(mems, dtype=np.float32)
    gamma = np.asarray(gamma, dtype=np.float32)
    beta = np.asarray(beta, dtype=np.float32)
    add_pos = bool(int(add_position))
    trivial = bool(np.all(gamma == 1.0) and np.all(beta == 0.0))

    nc = _get_program(add_pos, trivial)
    in_maps = _make_in_maps(dec_inp, pos_emb, mems, gamma, beta, add_pos, trivial)
    res = run_bass_kernel_spmd(nc, in_maps, list(range(BATCH)))
    return _unpack_out(res)
